# revision 11
# baseline (speedup 1.0000x reference)
"""e3nn-style 5x5x5 SAME conv3d ([2,32,32,32,32] -> [2,32,32,32,288]) on 8 trn2 cores.

Parity-folded implicit GEMM (fp16). The synthesized kernel k(t) satisfies
k(-t) = (-1)^l k(t) per l-block, so the conv splits into an even part (l=0,2:
192 out-ch) consuming P+ = x[p+t] + x[p-t] and an odd part (l=1: 96 out-ch)
consuming P- = x[p+t] - x[p-t]. The radial basis vanishes for |t| > 2.5, so
only 80 of 125 taps carry weight (40 +/- pairs plus the w_lin center).

  - EVEN part: fully folded into 11 K-tiles, each (4 z-level groups
    ζ=-2..1 x 32 c_in) on a P+ presum slab: one tile per bar family ((0,0);
    (1,0),(0,1),(1,1),(1,-1) with 5 slots; (2,0),(0,2),(2,1),(2,-1),(1,2),
    (1,-2) with 3); the five-slot families' dz=+2 pairs are stuffed as
    mirrored reps into the dead group-0 slots of the three-slot tiles.
  - ODD part: DVE-balanced hybrid — the six 3-slot families fold onto P-
    tiles (6 x 96 cols); the rest runs unfolded on the raw slabs (13 A-bars
    + 3 B-tiles covering the dz=+2 plane, 16 x 96 cols).
  - Per output block: 11x192 + 22x96 accumulated columns (vs 32x288 dense).
  - P+/P- slabs are built on-device by DVE add/sub from two host slab
    replicas: A (z-shift ζ_r = r-2 per group) for x[q+t], A2 (shift -ζ_r)
    for x[q-t]; (dx,dy) are window offsets (A2 windows mirrored). Built per
    output x-plane, double-buffered, overlapping the PE.
  - Conv weights are synthesized on device from the tiny radial weights
    (fp32r matmuls against host EY constants, one per (l, v)); the center
    tap uses w_lin/(2*fan) since P+ at the center slot equals 2x.
  - Output is written as fp16; the host widens to fp32.
"""

import numpy as np

try:
    import concourse.bass as bass  # noqa: F401
except ImportError:
    import sys

    sys.path.insert(0, "/opt/trn_rl_repo")

import concourse.mybir as mybir
import concourse.tile as tile
from concourse import bacc
from concourse.bass_utils import run_bass_kernel_spmd

F32 = mybir.dt.float32
F32R = mybir.dt.float32r
F16 = mybir.dt.float16

GRID = 32
CIN = 32
COUT = 288
NRB = 8
XPER = 8
XS = 12  # slab x extent (XPER + 2*2 halo)
FAN = float(np.sqrt(32.0))
NVOX = 125.0

# tile families: (dx, dy) in centered coords
FAM0 = (0, 0)
FAM5 = [(1, 0), (0, 1), (1, 1), (1, -1)]
FAM3 = [(2, 0), (0, 2), (2, 1), (2, -1), (1, 2), (1, -2)]
FAMS = [FAM0] + FAM5 + FAM3  # tiles 0..10
NT = len(FAMS)  # 11 tiles per parity
NTP = 12  # padded tile count for even-N fp32r synthesis matmuls

# per-parity column layouts
NE = 192  # even: l=0 (32) + l=2 (160)
NO = 96  # odd: l=1
A_XSTR = 36 * 32
SLAB_F = XS * A_XSTR  # per-replica slab elems (12 x-planes)
B_XSTR = 32 * 32
SLABB_F = XS * B_XSTR

# odd part: mostly unfolded (grid coords 0..4) — A-bars + 3 B-tiles — except
# four FAM3 families that fold cleanly (their dz=+2 taps are zero-radius):
# their 8 bars drop from the unfolded set and 4 P- folded tiles replace them.
O_FOLD = [(2, 0), (0, 2), (2, 1), (2, -1), (1, 2), (1, -2)]  # centered reps
O_FOLD5 = []  # five-slot families stay unfolded (folding them goes DVE-bound)
_FOLD_BARS = {
    (dx + 2, dy + 2)
    for (cx, cy) in O_FOLD + O_FOLD5
    for (dx, dy) in ((cx, cy), (-cx, -cy))
}
O_BARS = [
    (dx, dy)
    for dx in range(5)
    for dy in range(5)
    if (dx - 2) ** 2 + (dy - 2) ** 2 <= 6.25 and (dx, dy) not in _FOLD_BARS
]
O_NA = len(O_BARS)  # 5
O_BDXS = [1, 2, 3]
O_NKT = O_NA + len(O_BDXS) + len(O_FOLD) + len(O_FOLD5)  # 16


def _o_tap(b, r):
    """Rep tap (flat idx) for odd K-tile b, group r (grid), or None if dead."""
    if b < O_NA:
        dx, dy = O_BARS[b]
        return (dx * 5 + dy) * 5 + r
    if b < O_NA + len(O_BDXS):
        dx = O_BDXS[b - O_NA]
        # dz=+2 taps absorbed by the folded FAM5 pairs are zeroed here
        if any((dx, r) == (2 + cx, 2 + cy) for cx, cy in O_FOLD5):
            return None
        return (dx * 5 + r) * 5 + 4
    j = b - O_NA - len(O_BDXS)
    if j < len(O_FOLD):  # folded FAM3 tiles
        if r == 0:
            if j < len(O_FOLD5):  # stuffed mirrored FAM5 rep
                cx, cy = O_FOLD5[j]
                return ((2 - cx) * 5 + (2 - cy)) * 5 + 0
            return None
        cx, cy = O_FOLD[j]
        return ((cx + 2) * 5 + (cy + 2)) * 5 + r
    cx, cy = O_FOLD5[j - len(O_FOLD)]  # folded FAM5: all 4 groups live
    return ((cx + 2) * 5 + (cy + 2)) * 5 + r


def _slots(b):
    """[(group r, rep tap (dx,dy,dz) centered)] for K-tile b (None = center).

    FAM5 tiles hold their family's dz=-2..1 reps; each family's 5th pair
    (dz=+2 level) is stuffed, as the mirrored rep (-dx,-dy,-2), into the
    dead group-0 slot of FAM3 tile 5+i (shift -2 lives on partitions 0:32).
    """
    fam = FAMS[b]
    if fam == FAM0:
        return [(0, (0, 0, -2)), (2, None), (3, (0, 0, 1))]
    dx, dy = fam
    if fam in FAM5:
        return [(r, (dx, dy, r - 2)) for r in range(4)]
    slots = [(r, (dx, dy, r - 2)) for r in (1, 2, 3)]
    i = b - 1 - len(FAM5)
    if i < len(FAM5):
        sx, sy = FAM5[i]
        slots.append((0, (-sx, -sy, -2)))
    return slots


def _host_consts():
    """EY constants for the folded weight synthesis, per parity.

    ey[k-row (r*8+k), col]: col layouts: even: per-tile 12 blocks of
    (l0: 1) then (l2: 5) grouped l-major across tiles:
      l=0: cols [0, NT); l=2: cols [NT, NT+5*NT)
    odd (l=1): cols [0, 3*NT).
    """
    c = np.arange(-2.0, 3.0)
    lat = np.stack(np.meshgrid(c, c, c, indexing="ij"), axis=-1).reshape(125, 3)
    rad = np.linalg.norm(lat, axis=-1)
    u = lat / np.where(rad == 0.0, 1.0, rad)[:, None]
    ux, uy, uz = u[:, 0], u[:, 1], u[:, 2]

    y0 = np.ones((125, 1))
    y1 = np.sqrt(3.0) * np.stack([uy, uz, ux], axis=-1)
    y2 = np.stack(
        [
            np.sqrt(15.0) * ux * uy,
            np.sqrt(15.0) * uy * uz,
            (np.sqrt(5.0) / 2.0) * (2.0 * uz**2 - ux**2 - uy**2),
            np.sqrt(15.0) * ux * uz,
            (np.sqrt(15.0) / 2.0) * (ux**2 - uy**2),
        ],
        axis=-1,
    )
    ys = (y0, y1, y2)

    values = np.linspace(0.0, 2.5, NRB + 2)
    step = values[1] - values[0]
    values = values[1:-1]
    d = (rad[:, None] - values) / step

    def sus(x):
        return np.where(x > 0.0, np.exp(-1.0 / np.where(x > 0.0, x, 1.0)), 0.0)

    emb = 1.14136 * np.exp(2.0) * sus(d + 1.0) * sus(1.0 - d)  # [125, 8]
    emb = emb / (NVOX * FAN)

    def tap_idx(t):
        return ((t[0] + 2) * 5 + (t[1] + 2)) * 5 + (t[2] + 2)

    ey_e = np.zeros((32, NTP + 5 * NTP), np.float32)
    for b in range(NT):
        for r, t in _slots(b):
            if t is None:
                continue
            ti = tap_idx(t)
            rows = slice(r * 8, r * 8 + 8)
            ey_e[rows, b] = emb[ti] * ys[0][ti, 0]
            ey_e[rows, NTP + b * 5 : NTP + b * 5 + 5] = emb[ti, :, None] * ys[2][ti]
    # odd part: unfolded A/B tiles + 4 folded FAM3 tiles
    ey_o = np.zeros((32, 3 * O_NKT), np.float32)
    for b in range(O_NKT):
        for r in range(4):
            ti = _o_tap(b, r)
            if ti is None:
                continue
            rows = slice(r * 8, r * 8 + 8)
            ey_o[rows, b * 3 : b * 3 + 3] = emb[ti, :, None] * ys[1][ti]
    return ey_e, ey_o


def _build_nc(repeat=1):
    nc = bacc.Bacc("TRN2", target_bir_lowering=False, debug=False)

    a_d = nc.dram_tensor("slaba", [128, SLAB_F], F16, kind="ExternalInput")
    a2_d = nc.dram_tensor("slaba2", [128, SLAB_F], F16, kind="ExternalInput")
    b_d = nc.dram_tensor("slabb", [128, SLABB_F], F16, kind="ExternalInput")
    w_d = [
        nc.dram_tensor(f"w{l}t", [NRB, 32, 32], F32R, kind="ExternalInput")
        for l in range(3)
    ]
    wlin_d = nc.dram_tensor("wlin", [32, 32], F32, kind="ExternalInput")
    eye_d = nc.dram_tensor("eye", [32, 6 * NTP], F32R, kind="ExternalInput")
    eyo_d = nc.dram_tensor("eyo", [32, 3 * O_NKT], F32R, kind="ExternalInput")
    out_d = nc.dram_tensor("out", [XPER * 32 * 32, COUT], F16, kind="ExternalOutput")

    with tile.TileContext(nc) as tc:
        with (
            tc.tile_pool(name="wpool", bufs=1) as wpool,
            tc.tile_pool(name="rall", bufs=1) as rall_pool,
            tc.tile_pool(name="slab", bufs=1) as slab_pool,
            tc.tile_pool(name="pslab", bufs=2) as pslab_pool,
            tc.tile_pool(name="stage", bufs=4) as stage_pool,
            tc.tile_pool(name="ps", bufs=8, space="PSUM") as ps_pool,
        ):
            # --- tiny inputs ---
            eye_sb = wpool.tile([32, 6 * NTP], F32R, tag="eye", name="eye_sb")
            nc.sync.dma_start(eye_sb[:], eye_d[:])
            eyo_sb = wpool.tile([32, 3 * O_NKT], F32R, tag="eyo", name="eyo_sb")
            nc.sync.dma_start(eyo_sb[:], eyo_d[:])
            wlin_sb = wpool.tile([128, 32], F32, tag="wlin", name="wlin_sb")
            nc.sync.dma_start(wlin_sb[64:96, :], wlin_d[:])

            # Block-diagonal stationary: BD_l[8r+k, v*128 + 32r+u] = w_l[k,u,v]
            bd_sb = []
            for l in range(3):
                bd_l = wpool.tile([32, 32 * 128], F32R, tag=f"bd{l}", name=f"bd{l}")
                nc.vector.memset(bd_l[:].bitcast(F32), 0.0)
                bd3 = bd_l.rearrange("p (v q) -> p v q", q=128)
                for r in range(4):
                    nc.sync.dma_start(
                        bd3[8 * r : 8 * r + 8, :, 32 * r : 32 * r + 32], w_d[l][:]
                    )
                bd_sb.append(bd_l)

            # --- source slabs (A: z-shift r-2; A2: z-shift 2-r; B: y-shift) ---
            a_sb, a2_sb, b_sb = [], [], []
            for xpl in range(XS):
                sl = slice(xpl * A_XSTR, (xpl + 1) * A_XSTR)
                t_a = slab_pool.tile([128, 36, 32], F16, tag=f"A{xpl}", name=f"a{xpl}")
                nc.sync.dma_start(t_a.rearrange("p y z -> p (y z)"), a_d[:, sl])
                a_sb.append(t_a)
                t_b = slab_pool.tile([128, 36, 32], F16, tag=f"A2{xpl}", name=f"a2{xpl}")
                nc.sync.dma_start(t_b.rearrange("p y z -> p (y z)"), a2_d[:, sl])
                a2_sb.append(t_b)
                if 1 <= xpl <= 10:  # B windows only use planes 1..10
                    t_c = slab_pool.tile(
                        [128, B_XSTR], F16, tag=f"B{xpl}", name=f"b{xpl}"
                    )
                    nc.sync.dma_start(
                        t_c[:], b_d[:, xpl * B_XSTR : (xpl + 1) * B_XSTR]
                    )
                    b_sb.append(t_c)
                else:
                    b_sb.append(None)
            a_fl = [t.rearrange("p y z -> p (y z)") for t in a_sb]

            # --- conv-weight synthesis ---
            # r_e[(r,u), (b, 192)]: cols per tile: l0 v (32) | l2 (v,m) (160)
            # r_o[(r,u), (b, 96)]: cols per tile: l1 (v,m)
            r_e = rall_pool.tile([128, NT * NE], F16, tag="re", name="r_e")
            r_o = rall_pool.tile([128, O_NKT * NO], F16, tag="ro", name="r_o")
            re_v = r_e.rearrange("p (b c) -> p b c", c=NE)
            ro_v = r_o.rearrange("p (b c) -> p b c", c=NO)
            for v in range(32):
                ps0 = ps_pool.tile([128, NTP], F32, tag="ps", name="ps_s0")
                nc.tensor.matmul(
                    ps0[:, :],
                    bd_sb[0][:, 128 * v : 128 * (v + 1)],
                    eye_sb[:, 0:NTP],
                    start=True,
                    stop=True,
                )
                nc.vector.tensor_copy(re_v[:, :, v], ps0[:, 0:NT])
                ps2 = ps_pool.tile([128, 5 * NTP], F32, tag="ps", name="ps_s2")
                nc.tensor.matmul(
                    ps2[:, :],
                    bd_sb[2][:, 128 * v : 128 * (v + 1)],
                    eye_sb[:, NTP : 6 * NTP],
                    start=True,
                    stop=True,
                )
                nc.vector.tensor_copy(
                    re_v[:, :, 32 + 5 * v : 32 + 5 * (v + 1)],
                    ps2.rearrange("p (b m) -> p b m", m=5)[:, 0:NT, :],
                )
                ps1 = ps_pool.tile([128, 3 * O_NKT], F32, tag="ps", name="ps_s1")
                nc.tensor.matmul(
                    ps1[:, :],
                    bd_sb[1][:, 128 * v : 128 * (v + 1)],
                    eyo_sb[:, :],
                    start=True,
                    stop=True,
                )
                nc.vector.tensor_copy(
                    ro_v[:, :, 3 * v : 3 * (v + 1)],
                    ps1.rearrange("p (b m) -> p b m", m=3),
                )
            # center tap: tile 0 (fam (0,0)), group r=2, l0 block; P+ = 2x there
            nc.scalar.mul(re_v[64:96, 0, 0:32], wlin_sb[64:96, :], 0.5 / FAN)

            # --- folded conv: per x-plane build P+/P- slabs, then 8 blocks ---
            def build_pslabs(xo):
                """DVE-build P+ [(4 z-groups, 32u), 32y, 32z] per even K-tile.

                Full-width ops when all groups share the tile's family;
                stuffed FAM3 tiles overwrite group 0 with the mirrored rep.
                """
                pe = []
                for b in range(NT):
                    t_pe = pslab_pool.tile(
                        [128, 32, 32], F16, tag=f"pe{b}", name=f"pe{b}_{xo}"
                    )
                    dx, dy = FAMS[b]
                    i = b - 1 - len(FAM5)
                    stuffed = 0 <= i < len(FAM5)
                    in0 = a_sb[xo + 2 + dx][:, 2 + dy : 34 + dy, :]
                    in1 = a2_sb[xo + 2 - dx][:, 2 - dy : 34 - dy, :]
                    nc.vector.tensor_tensor(
                        t_pe[:], in0, in1, mybir.AluOpType.add
                    )
                    if stuffed:
                        # overwrite group 0 with the stuffed mirrored rep
                        sx, sy = FAM5[i]  # stuffed rep (-sx, -sy, -2)
                        in0 = a_sb[xo + 2 - sx][0:32, 2 - sy : 34 - sy, :]
                        in1 = a2_sb[xo + 2 + sx][0:32, 2 + sy : 34 + sy, :]
                        nc.vector.tensor_tensor(
                            t_pe[0:32], in0, in1, mybir.AluOpType.add
                        )
                    pe.append(t_pe.rearrange("p y z -> p (y z)"))
                po = []
                for j, (cx, cy) in enumerate(O_FOLD + O_FOLD5):
                    t_po = pslab_pool.tile(
                        [128, 32, 32], F16, tag=f"po{j}", name=f"po{j}_{xo}"
                    )
                    in0 = a_sb[xo + 2 + cx][:, 2 + cy : 34 + cy, :]
                    in1 = a2_sb[xo + 2 - cx][:, 2 - cy : 34 - cy, :]
                    # route 4 of the 6 folded odd builds to the idle GpSimd
                    # (Pool) engine to take load off the DVE bottleneck
                    eng = nc.gpsimd if j >= 2 else nc.vector
                    eng.tensor_tensor(
                        t_po[:], in0, in1, mybir.AluOpType.subtract
                    )
                    if j < len(O_FOLD5):
                        # overwrite group 0 with the stuffed mirrored FAM5 rep
                        sx, sy = O_FOLD5[j]
                        in0 = a_sb[xo + 2 - sx][0:32, 2 - sy : 34 - sy, :]
                        in1 = a2_sb[xo + 2 + sx][0:32, 2 + sy : 34 + sy, :]
                        nc.vector.tensor_tensor(
                            t_po[0:32], in0, in1, mybir.AluOpType.subtract
                        )
                    po.append(t_po.rearrange("p y z -> p (y z)"))
                return pe, po

            def conv_pass():
                for xo in range(XPER):
                    pe, po = build_pslabs(xo)
                    for yb in range(8):
                        w0 = 128 * yb
                        ps = ps_pool.tile([128, NE + NO], F32, tag="ps", name="ps")
                        ps_e = ps[:, 0:NE]
                        ps_o = ps[:, NE : NE + NO]

                        def mm_e(b):
                            nc.tensor.matmul(
                                ps_e[:, :],
                                pe[b][:, w0 : w0 + 128],
                                r_e[:, b * NE : (b + 1) * NE],
                                start=(b == 0),
                                stop=(b == NT - 1),
                                skip_group_check=True,
                            )

                        def mm_o(b):
                            if b < O_NA:
                                gdx, gdy = O_BARS[b]
                                win = a_fl[xo + gdx][
                                    :, (4 * yb + gdy) * 32 : (4 * yb + gdy) * 32 + 128
                                ]
                            elif b < O_NA + len(O_BDXS):
                                gdx = O_BDXS[b - O_NA]
                                win = b_sb[xo + gdx][:, w0 : w0 + 128]
                            else:
                                win = po[b - O_NA - len(O_BDXS)][:, w0 : w0 + 128]
                            # ps_o shares the even group's PSUM bank: never
                            # start=True here (it would clear the whole bank);
                            # the block's first even matmul cleared it, so the
                            # first odd matmul per element overwrites via the
                            # has_written bit.
                            nc.tensor.matmul(
                                ps_o[:, :],
                                win,
                                r_o[:, b * NO : (b + 1) * NO],
                                start=False,
                                stop=(b == O_NKT - 1),
                                skip_group_check=True,
                            )

                        # interleave E and O so odd 96-col matmuls' 128-row
                        # weight loads hide under the wider even streams
                        oi = 0
                        for i in range(NT):
                            mm_e(i)
                            take = 2 if (O_NKT - oi) > (NT - 1 - i) else 1
                            for _ in range(min(take, O_NKT - oi)):
                                mm_o(oi)
                                oi += 1
                        while oi < O_NKT:
                            mm_o(oi)
                            oi += 1
                        # out cols are [l0 | l2 | l1] (even block then odd);
                        # the host permutes back — single widening copy
                        stg = stage_pool.tile([128, COUT], F16, tag="stg", name="stg")
                        nc.scalar.copy(stg[:, :], ps[:, :])
                        row = xo * 1024 + yb * 128
                        nc.sync.dma_start(out_d[row : row + 128, :], stg[:])

            import os as _os

            _unroll = int(_os.environ.get("UNROLL", "1"))
            if repeat == 1:
                conv_pass()
            else:
                with tc.For_i(0, repeat // _unroll):
                    for _ in range(_unroll):
                        conv_pass()

    nc.compile()
    return nc


def _shard_inputs(x, w0, w1, w2, w_lin):
    ey_e, ey_o = _host_consts()
    wts = [
        np.ascontiguousarray(w.transpose(0, 2, 1)).astype(np.float32)
        for w in (w0, w1, w2)
    ]
    w_lin = np.ascontiguousarray(w_lin).astype(np.float32)
    in_maps = []
    for core in range(8):
        bb, xi = divmod(core, 4)
        x0 = xi * XPER
        pp = np.zeros((CIN, XS, 36, 36), np.float32)
        glo, ghi = x0 - 2, x0 + XPER + 2
        slo, shi = max(glo, 0), min(ghi, GRID)
        pp[:, slo - glo : shi - glo, 2:34, 2:34] = x[bb, slo:shi].transpose(3, 0, 1, 2)
        p4a = np.stack([pp[:, :, :, r : r + 32] for r in range(4)], axis=0)
        p4a2 = np.stack([pp[:, :, :, 4 - r : 36 - r] for r in range(4)], axis=0)
        p4b = np.stack([pp[:, :, r : r + 32, 4:36] for r in range(4)], axis=0)
        in_maps.append(
            {
                "slaba": np.ascontiguousarray(p4a).reshape(128, -1).astype(np.float16),
                "slaba2": np.ascontiguousarray(p4a2)
                .reshape(128, -1)
                .astype(np.float16),
                "slabb": np.ascontiguousarray(p4b).reshape(128, -1).astype(np.float16),
                "w0t": wts[0],
                "w1t": wts[1],
                "w2t": wts[2],
                "wlin": w_lin,
                "eye": ey_e,
                "eyo": ey_o,
            }
        )
    return in_maps


_NC = None


def _run(x, w0, w1, w2, w_lin, **spmd_kwargs):
    global _NC
    if _NC is None:
        _NC = _build_nc()
    in_maps = _shard_inputs(
        np.asarray(x, np.float32),
        np.asarray(w0, np.float32),
        np.asarray(w1, np.float32),
        np.asarray(w2, np.float32),
        np.asarray(w_lin, np.float32),
    )
    res = run_bass_kernel_spmd(_NC, in_maps, core_ids=list(range(8)), **spmd_kwargs)
    out = np.empty((2, GRID, GRID, GRID, COUT), np.float32)
    for core in range(8):
        bb, xi = divmod(core, 4)
        r = res.results[core]["out"].astype(np.float32).reshape(XPER, GRID, GRID, COUT)
        dst = out[bb, xi * XPER : (xi + 1) * XPER]
        # device col layout is [l0 (32) | l2 (160) | l1 (96)]; permute back
        dst[..., 0:32] = r[..., 0:32]
        dst[..., 32:128] = r[..., 192:288]
        dst[..., 128:288] = r[..., 32:192]
    return out, res


def kernel(x, w0, w1, w2, w_lin):
    out, _ = _run(x, w0, w1, w2, w_lin)
    return out



# revision 41
# speedup vs baseline: 1.1837x; 1.1837x over previous
"""e3nn-style 5x5x5 SAME conv3d ([2,32,32,32,32] -> [2,32,32,32,288]) on 8 trn2 cores.

Parity-folded implicit GEMM (fp16). The synthesized kernel k(t) satisfies
k(-t) = (-1)^l k(t) per l-block, so the conv splits into an even part (l=0,2:
192 out-ch) consuming P+ = x[p+t] + x[p-t] and an odd part (l=1: 96 out-ch)
consuming P- = x[p+t] - x[p-t]. The radial basis vanishes for |t| > 2.5, so
only 80 of 125 taps carry weight (40 +/- pairs plus the w_lin center).

  - EVEN part: fully folded into 11 K-tiles, each (4 z-level groups
    ζ=-2..1 x 32 c_in) on a P+ presum slab: one tile per bar family ((0,0);
    (1,0),(0,1),(1,1),(1,-1) with 5 slots; (2,0),(0,2),(2,1),(2,-1),(1,2),
    (1,-2) with 3); the five-slot families' dz=+2 pairs are stuffed as
    mirrored reps into the dead group-0 slots of the three-slot tiles.
  - ODD part: DVE-balanced hybrid — the six 3-slot families (plus NFOLD5
    five-slot families) fold onto P- tiles; the rest runs unfolded on the
    raw slabs (A-bars + 3 B-tiles covering the dz=+2 plane), 96 cols each.
  - P+/P- slabs are built on-device by DVE add/sub from two host slab
    replicas: A (z-shift ζ_r = r-2 per group) for x[q+t], A2 (shift -ζ_r)
    for x[q-t]; (dx,dy) are window offsets (A2 windows mirrored). Built per
    output x-plane, double-buffered, overlapping the PE. Measured walls per
    pass: PE pipeline ~114us, DVE fold pipeline ~101us (Pool/GpSimd TT is
    ~8x slower than DVE on HW, Activation cannot do two-tensor adds).
  - Per block both parities accumulate into ONE PSUM bank ([l0|l2|l1] col
    layout, odd matmuls never set start), evacuated by a single widening
    Activation copy + per-block DMA (plane-batched DMA measured slower).
  - Conv weights are synthesized on device from the tiny radial weights
    (fp32r matmuls against host EY constants, one per (l, v)); the center
    tap uses w_lin/(2*fan) since P+ at the center slot equals 2x.
  - Output is written as fp16 [l0|l2|l1]; the host widens and permutes.
"""

import os as _os

import numpy as np

try:
    import concourse.bass as bass  # noqa: F401
except ImportError:
    import sys

    sys.path.insert(0, "/opt/trn_rl_repo")

# perf-probe knobs (default = production config)
_POOL_TT = int(_os.environ.get("POOL_TT", "0"))  # folded-odd TTs on GpSimd
_SKIP_TT = int(_os.environ.get("SKIP_TT", "0"))  # shrink P± builds to 1 y-row
_SKIP_MM = int(_os.environ.get("SKIP_MM", "0"))  # only 1 matmul per block
_PSLAB_BUFS = int(_os.environ.get("PSLAB_BUFS", "2"))  # P± double-buffer depth

import concourse.mybir as mybir
import concourse.tile as tile
from concourse import bacc
from concourse.bass_utils import run_bass_kernel_spmd

F32 = mybir.dt.float32
F32R = mybir.dt.float32r
F16 = mybir.dt.float16

GRID = 32
CIN = 32
COUT = 288
NRB = 8
XPER = 8
XS = 12  # slab x extent (XPER + 2*2 halo)
FAN = float(np.sqrt(32.0))
NVOX = 125.0

# tile families: (dx, dy) in centered coords
FAM0 = (0, 0)
FAM5 = [(1, 0), (0, 1), (1, 1), (1, -1)]
FAM3 = [(2, 0), (0, 2), (2, 1), (2, -1), (1, 2), (1, -2)]
FAMS = [FAM0] + FAM5 + FAM3  # tiles 0..10
NT = len(FAMS)  # 11 tiles per parity
NTP = 12  # padded tile count for even-N fp32r synthesis matmuls

# per-parity column layouts
NE = 192  # even: l=0 (32) + l=2 (160)
NO = 96  # odd: l=1
A_XSTR = 36 * 32
SLAB_F = XS * A_XSTR  # per-replica slab elems (12 x-planes)
B_XSTR = 32 * 32
SLABB_F = XS * B_XSTR

# odd part: mostly unfolded (grid coords 0..4) — A-bars + 3 B-tiles — except
# four FAM3 families that fold cleanly (their dz=+2 taps are zero-radius):
# their 8 bars drop from the unfolded set and 4 P- folded tiles replace them.
O_FOLD = [(2, 0), (0, 2), (2, 1), (2, -1), (1, 2), (1, -2)]  # centered reps
# five-slot families folded onto P- (each adds 2 DVE ops/plane, removes one
# 96-col matmul/block); NFOLD5 picks the PE/DVE balance point
_NFOLD5 = int(_os.environ.get("NFOLD5", "0"))
O_FOLD5 = [(1, 0), (0, 1), (1, 1), (1, -1)][:_NFOLD5]
_FOLD_BARS = {
    (dx + 2, dy + 2)
    for (cx, cy) in O_FOLD + O_FOLD5
    for (dx, dy) in ((cx, cy), (-cx, -cy))
}
O_BARS = [
    (dx, dy)
    for dx in range(5)
    for dy in range(5)
    if (dx - 2) ** 2 + (dy - 2) ** 2 <= 6.25 and (dx, dy) not in _FOLD_BARS
]
O_NA = len(O_BARS)  # 5
O_BDXS = [1, 2, 3]
O_NKT = O_NA + len(O_BDXS) + len(O_FOLD) + len(O_FOLD5)  # 16
O_EYW = 3 * O_NKT + (3 * O_NKT) % 2  # even-N pad for fp32r synthesis matmul


def _o_tap(b, r):
    """Rep tap (flat idx) for odd K-tile b, group r (grid), or None if dead."""
    if b < O_NA:
        dx, dy = O_BARS[b]
        return (dx * 5 + dy) * 5 + r
    if b < O_NA + len(O_BDXS):
        dx = O_BDXS[b - O_NA]
        # dz=+2 taps absorbed by the folded FAM5 pairs are zeroed here:
        # (cx,cy,+2) joins the zeta=+2 pair, (-cx,-cy,+2) the zeta=-2 pair
        if any(
            (dx, r) in ((2 + cx, 2 + cy), (2 - cx, 2 - cy)) for cx, cy in O_FOLD5
        ):
            return None
        return (dx * 5 + r) * 5 + 4
    j = b - O_NA - len(O_BDXS)
    if j < len(O_FOLD):  # folded FAM3 tiles
        if r == 0:
            if j < len(O_FOLD5):  # stuffed mirrored FAM5 rep
                cx, cy = O_FOLD5[j]
                return ((2 - cx) * 5 + (2 - cy)) * 5 + 0
            return None
        cx, cy = O_FOLD[j]
        return ((cx + 2) * 5 + (cy + 2)) * 5 + r
    cx, cy = O_FOLD5[j - len(O_FOLD)]  # folded FAM5: all 4 groups live
    return ((cx + 2) * 5 + (cy + 2)) * 5 + r


def _slots(b):
    """[(group r, rep tap (dx,dy,dz) centered)] for K-tile b (None = center).

    FAM5 tiles hold their family's dz=-2..1 reps; each family's 5th pair
    (dz=+2 level) is stuffed, as the mirrored rep (-dx,-dy,-2), into the
    dead group-0 slot of FAM3 tile 5+i (shift -2 lives on partitions 0:32).
    """
    fam = FAMS[b]
    if fam == FAM0:
        return [(0, (0, 0, -2)), (2, None), (3, (0, 0, 1))]
    dx, dy = fam
    if fam in FAM5:
        return [(r, (dx, dy, r - 2)) for r in range(4)]
    slots = [(r, (dx, dy, r - 2)) for r in (1, 2, 3)]
    i = b - 1 - len(FAM5)
    if i < len(FAM5):
        sx, sy = FAM5[i]
        slots.append((0, (-sx, -sy, -2)))
    return slots


def _host_consts():
    """EY constants for the folded weight synthesis, per parity.

    ey[k-row (r*8+k), col]: col layouts: even: per-tile 12 blocks of
    (l0: 1) then (l2: 5) grouped l-major across tiles:
      l=0: cols [0, NT); l=2: cols [NT, NT+5*NT)
    odd (l=1): cols [0, 3*NT).
    """
    c = np.arange(-2.0, 3.0)
    lat = np.stack(np.meshgrid(c, c, c, indexing="ij"), axis=-1).reshape(125, 3)
    rad = np.linalg.norm(lat, axis=-1)
    u = lat / np.where(rad == 0.0, 1.0, rad)[:, None]
    ux, uy, uz = u[:, 0], u[:, 1], u[:, 2]

    y0 = np.ones((125, 1))
    y1 = np.sqrt(3.0) * np.stack([uy, uz, ux], axis=-1)
    y2 = np.stack(
        [
            np.sqrt(15.0) * ux * uy,
            np.sqrt(15.0) * uy * uz,
            (np.sqrt(5.0) / 2.0) * (2.0 * uz**2 - ux**2 - uy**2),
            np.sqrt(15.0) * ux * uz,
            (np.sqrt(15.0) / 2.0) * (ux**2 - uy**2),
        ],
        axis=-1,
    )
    ys = (y0, y1, y2)

    values = np.linspace(0.0, 2.5, NRB + 2)
    step = values[1] - values[0]
    values = values[1:-1]
    d = (rad[:, None] - values) / step

    def sus(x):
        return np.where(x > 0.0, np.exp(-1.0 / np.where(x > 0.0, x, 1.0)), 0.0)

    emb = 1.14136 * np.exp(2.0) * sus(d + 1.0) * sus(1.0 - d)  # [125, 8]
    emb = emb / (NVOX * FAN)

    def tap_idx(t):
        return ((t[0] + 2) * 5 + (t[1] + 2)) * 5 + (t[2] + 2)

    ey_e = np.zeros((32, NTP + 5 * NTP), np.float32)
    for b in range(NT):
        for r, t in _slots(b):
            if t is None:
                continue
            ti = tap_idx(t)
            rows = slice(r * 8, r * 8 + 8)
            ey_e[rows, b] = emb[ti] * ys[0][ti, 0]
            ey_e[rows, NTP + b * 5 : NTP + b * 5 + 5] = emb[ti, :, None] * ys[2][ti]
    # odd part: unfolded A/B tiles + 4 folded FAM3 tiles
    ey_o = np.zeros((32, O_EYW), np.float32)
    for b in range(O_NKT):
        for r in range(4):
            ti = _o_tap(b, r)
            if ti is None:
                continue
            rows = slice(r * 8, r * 8 + 8)
            ey_o[rows, b * 3 : b * 3 + 3] = emb[ti, :, None] * ys[1][ti]
    return ey_e, ey_o


def _build_nc(repeat=1):
    nc = bacc.Bacc("TRN2", target_bir_lowering=False, debug=False)

    a_d = nc.dram_tensor("slaba", [128, SLAB_F], F16, kind="ExternalInput")
    a2_d = nc.dram_tensor("slaba2", [128, SLAB_F], F16, kind="ExternalInput")
    b_d = nc.dram_tensor("slabb", [128, SLABB_F], F16, kind="ExternalInput")
    w_d = [
        nc.dram_tensor(f"w{l}t", [NRB, 32, 32], F32R, kind="ExternalInput")
        for l in range(3)
    ]
    wlin_d = nc.dram_tensor("wlin", [32, 32], F32, kind="ExternalInput")
    eye_d = nc.dram_tensor("eye", [32, 6 * NTP], F32R, kind="ExternalInput")
    eyo_d = nc.dram_tensor("eyo", [32, O_EYW], F32R, kind="ExternalInput")
    out_d = nc.dram_tensor("out", [XPER * 32 * 32, COUT], F16, kind="ExternalOutput")

    with tile.TileContext(nc) as tc:
        with (
            tc.tile_pool(name="wpool", bufs=1) as wpool,
            tc.tile_pool(name="rall", bufs=1) as rall_pool,
            tc.tile_pool(name="slab", bufs=1) as slab_pool,
            tc.tile_pool(name="ps", bufs=8, space="PSUM") as ps_pool,
        ):
            # --- tiny inputs ---
            bd_scope = tc.tile_pool(name="bd", bufs=1)
            bd_pool = bd_scope.__enter__()
            eye_sb = bd_pool.tile([32, 6 * NTP], F32R, tag="eye", name="eye_sb")
            nc.sync.dma_start(eye_sb[:], eye_d[:])
            eyo_sb = bd_pool.tile([32, O_EYW], F32R, tag="eyo", name="eyo_sb")
            nc.sync.dma_start(eyo_sb[:], eyo_d[:])
            wlin_sb = wpool.tile([128, 32], F32, tag="wlin", name="wlin_sb")
            nc.sync.dma_start(wlin_sb[64:96, :], wlin_d[:])

            # Block-diagonal stationary: BD_l[8r+k, v*128 + 32r+u] = w_l[k,u,v]
            bd_sb = []
            for l in range(3):
                bd_l = bd_pool.tile([32, 32 * 128], F32R, tag=f"bd{l}", name=f"bd{l}")
                nc.vector.memset(bd_l[:].bitcast(F32), 0.0)
                bd3 = bd_l.rearrange("p (v q) -> p v q", q=128)
                for r in range(4):
                    nc.sync.dma_start(
                        bd3[8 * r : 8 * r + 8, :, 32 * r : 32 * r + 32], w_d[l][:]
                    )
                bd_sb.append(bd_l)

            # --- source slabs (A: z-shift r-2; A2: z-shift 2-r; B: y-shift) ---
            a_sb, a2_sb, b_sb = [], [], []
            for xpl in range(XS):
                sl = slice(xpl * A_XSTR, (xpl + 1) * A_XSTR)
                t_a = slab_pool.tile([128, 36, 32], F16, tag=f"A{xpl}", name=f"a{xpl}")
                nc.sync.dma_start(t_a.rearrange("p y z -> p (y z)"), a_d[:, sl])
                a_sb.append(t_a)
                t_b = slab_pool.tile([128, 36, 32], F16, tag=f"A2{xpl}", name=f"a2{xpl}")
                nc.sync.dma_start(t_b.rearrange("p y z -> p (y z)"), a2_d[:, sl])
                a2_sb.append(t_b)
                if 1 <= xpl <= 10:  # B windows only use planes 1..10
                    t_c = slab_pool.tile(
                        [128, B_XSTR], F16, tag=f"B{xpl}", name=f"b{xpl}"
                    )
                    nc.sync.dma_start(
                        t_c[:], b_d[:, xpl * B_XSTR : (xpl + 1) * B_XSTR]
                    )
                    b_sb.append(t_c)
                else:
                    b_sb.append(None)
            a_fl = [t.rearrange("p y z -> p (y z)") for t in a_sb]

            # --- conv-weight synthesis ---
            # r_e[(r,u), (b, 192)]: cols per tile: l0 v (32) | l2 (v,m) (160)
            # r_o[(r,u), (b, 96)]: cols per tile: l1 (v,m)
            r_e = rall_pool.tile([128, NT * NE], F16, tag="re", name="r_e")
            r_o = rall_pool.tile([128, O_NKT * NO], F16, tag="ro", name="r_o")
            re_v = r_e.rearrange("p (b c) -> p b c", c=NE)
            ro_v = r_o.rearrange("p (b c) -> p b c", c=NO)
            for v in range(32):
                ps0 = ps_pool.tile([128, NTP], F32, tag="ps", name="ps_s0")
                nc.tensor.matmul(
                    ps0[:, :],
                    bd_sb[0][:, 128 * v : 128 * (v + 1)],
                    eye_sb[:, 0:NTP],
                    start=True,
                    stop=True,
                )
                nc.vector.tensor_copy(re_v[:, :, v], ps0[:, 0:NT])
                ps2 = ps_pool.tile([128, 5 * NTP], F32, tag="ps", name="ps_s2")
                nc.tensor.matmul(
                    ps2[:, :],
                    bd_sb[2][:, 128 * v : 128 * (v + 1)],
                    eye_sb[:, NTP : 6 * NTP],
                    start=True,
                    stop=True,
                )
                nc.vector.tensor_copy(
                    re_v[:, :, 32 + 5 * v : 32 + 5 * (v + 1)],
                    ps2.rearrange("p (b m) -> p b m", m=5)[:, 0:NT, :],
                )
                ps1 = ps_pool.tile([128, O_EYW], F32, tag="ps", name="ps_s1")
                nc.tensor.matmul(
                    ps1[:, :],
                    bd_sb[1][:, 128 * v : 128 * (v + 1)],
                    eyo_sb[:, :],
                    start=True,
                    stop=True,
                )
                nc.vector.tensor_copy(
                    ro_v[:, :, 3 * v : 3 * (v + 1)],
                    ps1[:, 0 : 3 * O_NKT].rearrange("p (b m) -> p b m", m=3),
                )
            # center tap: tile 0 (fam (0,0)), group r=2, l0 block; P+ = 2x there
            nc.scalar.mul(re_v[64:96, 0, 0:32], wlin_sb[64:96, :], 0.5 / FAN)

            # free the synthesis scratch (48 KiB/partition) before the P±
            # pools open so deeper double-buffering fits
            bd_scope.__exit__(None, None, None)
            pslab_pool = tc.alloc_tile_pool(name="pslab", bufs=_PSLAB_BUFS)
            stage_pool = tc.alloc_tile_pool(name="stage", bufs=4)

            # --- folded conv: per x-plane build P+/P- slabs, then 8 blocks ---
            def build_pslabs(xo):
                """DVE-build P+ [(4 z-groups, 32u), 32y, 32z] per even K-tile.

                Full-width ops when all groups share the tile's family;
                stuffed FAM3 tiles overwrite group 0 with the mirrored rep.
                """
                yh = 1 if _SKIP_TT else 32  # probe: build only 1 y-row
                pe = []
                for b in range(NT):
                    t_pe = pslab_pool.tile(
                        [128, 32, 32], F16, tag=f"pe{b}", name=f"pe{b}_{xo}"
                    )
                    dx, dy = FAMS[b]
                    i = b - 1 - len(FAM5)
                    stuffed = 0 <= i < len(FAM5)
                    in0 = a_sb[xo + 2 + dx][:, 2 + dy : 2 + dy + yh, :]
                    in1 = a2_sb[xo + 2 - dx][:, 2 - dy : 2 - dy + yh, :]
                    nc.vector.tensor_tensor(
                        t_pe[:, 0:yh, :], in0, in1, mybir.AluOpType.add
                    )
                    if stuffed:
                        # overwrite group 0 with the stuffed mirrored rep
                        sx, sy = FAM5[i]  # stuffed rep (-sx, -sy, -2)
                        in0 = a_sb[xo + 2 - sx][0:32, 2 - sy : 2 - sy + yh, :]
                        in1 = a2_sb[xo + 2 + sx][0:32, 2 + sy : 2 + sy + yh, :]
                        nc.vector.tensor_tensor(
                            t_pe[0:32, 0:yh, :], in0, in1, mybir.AluOpType.add
                        )
                    pe.append(t_pe.rearrange("p y z -> p (y z)"))
                po = []
                for j, (cx, cy) in enumerate(O_FOLD + O_FOLD5):
                    t_po = pslab_pool.tile(
                        [128, 32, 32], F16, tag=f"po{j}", name=f"po{j}_{xo}"
                    )
                    in0 = a_sb[xo + 2 + cx][:, 2 + cy : 2 + cy + yh, :]
                    in1 = a2_sb[xo + 2 - cx][:, 2 - cy : 2 - cy + yh, :]
                    # route the last POOL_TT folded odd builds to the GpSimd
                    # (Pool) engine to take load off the DVE bottleneck
                    n_po = len(O_FOLD) + len(O_FOLD5)
                    eng = nc.gpsimd if j >= n_po - _POOL_TT else nc.vector
                    eng.tensor_tensor(
                        t_po[:, 0:yh, :], in0, in1, mybir.AluOpType.subtract
                    )
                    if j < len(O_FOLD5):
                        # overwrite group 0 with the stuffed mirrored FAM5 rep
                        sx, sy = O_FOLD5[j]
                        in0 = a_sb[xo + 2 - sx][0:32, 2 - sy : 2 - sy + yh, :]
                        in1 = a2_sb[xo + 2 + sx][0:32, 2 + sy : 2 + sy + yh, :]
                        nc.vector.tensor_tensor(
                            t_po[0:32, 0:yh, :], in0, in1, mybir.AluOpType.subtract
                        )
                    po.append(t_po.rearrange("p y z -> p (y z)"))
                return pe, po

            def conv_pass():
                for xo in range(XPER):
                    pe, po = build_pslabs(xo)
                    for yb in range(8):
                        w0 = 128 * yb
                        ps = ps_pool.tile([128, NE + NO], F32, tag="ps", name="ps")
                        ps_e = ps[:, 0:NE]
                        ps_o = ps[:, NE : NE + NO]

                        def mm_e(b):
                            nc.tensor.matmul(
                                ps_e[:, :],
                                pe[b][:, w0 : w0 + 128],
                                r_e[:, b * NE : (b + 1) * NE],
                                start=(b == 0),
                                stop=(b == NT - 1),
                                skip_group_check=True,
                            )

                        def mm_o(b):
                            if b < O_NA:
                                gdx, gdy = O_BARS[b]
                                win = a_fl[xo + gdx][
                                    :, (4 * yb + gdy) * 32 : (4 * yb + gdy) * 32 + 128
                                ]
                            elif b < O_NA + len(O_BDXS):
                                gdx = O_BDXS[b - O_NA]
                                win = b_sb[xo + gdx][:, w0 : w0 + 128]
                            else:
                                win = po[b - O_NA - len(O_BDXS)][:, w0 : w0 + 128]
                            # ps_o shares the even group's PSUM bank: never
                            # start=True here (it would clear the whole bank);
                            # the block's first even matmul cleared it, so the
                            # first odd matmul per element overwrites via the
                            # has_written bit.
                            nc.tensor.matmul(
                                ps_o[:, :],
                                win,
                                r_o[:, b * NO : (b + 1) * NO],
                                start=False,
                                stop=(b == O_NKT - 1),
                                skip_group_check=True,
                            )

                        if _SKIP_MM:
                            # probe: single matmul per block keeps the psum
                            # tile written but removes ~all PE work
                            nc.tensor.matmul(
                                ps[:, :],
                                pe[0][:, w0 : w0 + 128],
                                r_e[:, 0 : NE + NO],
                                start=True,
                                stop=True,
                                skip_group_check=True,
                            )
                        else:
                            # interleave E and O so odd 96-col matmuls' 128-row
                            # weight loads hide under the wider even streams
                            oi = 0
                            for i in range(NT):
                                mm_e(i)
                                take = 2 if (O_NKT - oi) > (NT - 1 - i) else 1
                                for _ in range(min(take, O_NKT - oi)):
                                    mm_o(oi)
                                    oi += 1
                            while oi < O_NKT:
                                mm_o(oi)
                                oi += 1
                        # out cols are [l0 | l2 | l1] (even block then odd);
                        # the host permutes back — single widening copy
                        stg = stage_pool.tile([128, COUT], F16, tag="stg", name="stg")
                        nc.scalar.copy(stg[:, :], ps[:, :])
                        row = xo * 1024 + yb * 128
                        nc.sync.dma_start(out_d[row : row + 128, :], stg[:])

            import os as _os

            _unroll = int(_os.environ.get("UNROLL", "1"))
            if repeat == 1:
                conv_pass()
            else:
                with tc.For_i(0, repeat // _unroll):
                    for _ in range(_unroll):
                        conv_pass()
            stage_pool.release()
            pslab_pool.release()

    nc.compile()
    return nc


def _shard_inputs(x, w0, w1, w2, w_lin):
    ey_e, ey_o = _host_consts()
    wts = [
        np.ascontiguousarray(w.transpose(0, 2, 1)).astype(np.float32)
        for w in (w0, w1, w2)
    ]
    w_lin = np.ascontiguousarray(w_lin).astype(np.float32)
    in_maps = []
    for core in range(8):
        bb, xi = divmod(core, 4)
        x0 = xi * XPER
        pp = np.zeros((CIN, XS, 36, 36), np.float32)
        glo, ghi = x0 - 2, x0 + XPER + 2
        slo, shi = max(glo, 0), min(ghi, GRID)
        pp[:, slo - glo : shi - glo, 2:34, 2:34] = x[bb, slo:shi].transpose(3, 0, 1, 2)
        p4a = np.stack([pp[:, :, :, r : r + 32] for r in range(4)], axis=0)
        p4a2 = np.stack([pp[:, :, :, 4 - r : 36 - r] for r in range(4)], axis=0)
        p4b = np.stack([pp[:, :, r : r + 32, 4:36] for r in range(4)], axis=0)
        in_maps.append(
            {
                "slaba": np.ascontiguousarray(p4a).reshape(128, -1).astype(np.float16),
                "slaba2": np.ascontiguousarray(p4a2)
                .reshape(128, -1)
                .astype(np.float16),
                "slabb": np.ascontiguousarray(p4b).reshape(128, -1).astype(np.float16),
                "w0t": wts[0],
                "w1t": wts[1],
                "w2t": wts[2],
                "wlin": w_lin,
                "eye": ey_e,
                "eyo": ey_o,
            }
        )
    return in_maps


_NC = None


def _run(x, w0, w1, w2, w_lin, **spmd_kwargs):
    global _NC
    if _NC is None:
        _NC = _build_nc()
    in_maps = _shard_inputs(
        np.asarray(x, np.float32),
        np.asarray(w0, np.float32),
        np.asarray(w1, np.float32),
        np.asarray(w2, np.float32),
        np.asarray(w_lin, np.float32),
    )
    res = run_bass_kernel_spmd(_NC, in_maps, core_ids=list(range(8)), **spmd_kwargs)
    out = np.empty((2, GRID, GRID, GRID, COUT), np.float32)
    for core in range(8):
        bb, xi = divmod(core, 4)
        r = res.results[core]["out"].astype(np.float32).reshape(XPER, GRID, GRID, COUT)
        dst = out[bb, xi * XPER : (xi + 1) * XPER]
        # device col layout is [l0 (32) | l2 (160) | l1 (96)]; permute back
        dst[..., 0:32] = r[..., 0:32]
        dst[..., 32:128] = r[..., 192:288]
        dst[..., 128:288] = r[..., 32:192]
    return out, res


def kernel(x, w0, w1, w2, w_lin):
    out, _ = _run(x, w0, w1, w2, w_lin)
    return out

